# revision 5
# baseline (speedup 1.0000x reference)
"""Trainium2 Bass kernel for nn_DendriteLayer (topk_masking).

Computation (see reference):
    h  = x @ w_in.T + b_in                    # [B, N_DEND]
    h3 = h.reshape(B, OUT_DIM, DPN)
    out[b,u] = h3[b,u,argmax_d h3[b,u,:]] * w_out[u, argmax_d] + b_out[u]

Sharding: OUT_DIM (and its DPN dendrite groups) split across 8 cores;
x replicated; no cross-core communication. Each core computes a
[B, OUT_DIM/8] slice of the output.

Device layout: batch on partitions, dendrites on the free dim, so the
per-unit max over DPN=16 consecutive dendrites is a free-dim segmented
reduce on the vector engine. w_in is pre-transposed on host to
[IN_DIM, N_DEND] so the contraction dim lands on partitions with
contiguous DMA rows.
"""

import numpy as np

import concourse.bass as bass
import concourse.mybir as mybir
from concourse import tile
from concourse.bass_utils import run_bass_kernel_spmd
from concourse.vector_clock import ScopedClock
from contextlib import ExitStack

# Problem shapes (hardcoded per contract).
B = 256          # batch
K = 1024         # in_dim
OUT_DIM = 2048
DPN = 16
N_CORES = 8
D_SH = (OUT_DIM // N_CORES) * DPN   # 4096 dendrites per core
U_SH = OUT_DIM // N_CORES           # 256 units per core
KT = K // 128                       # 8 k-tiles
DC_W = 512                          # dendrite chunk width (PSUM bank)
DC = D_SH // DC_W                   # 8 chunks
UC = DC_W // DPN                    # 32 units per chunk
NB = B // 128                       # 2 batch tiles
DT = mybir.dt.float32
AX = mybir.AxisListType.X


def _patch_tile_tail_drain():
    """Workaround: this container's walrus build rejects >2 semaphore
    waits on one InstDrain ("Too many sync wait commands"). Move the
    TileContext tail-drain waits onto individual SP NOPs (one wait
    each); SP program order keeps the drain equivalent."""
    if getattr(tile.TileContext, "_ant_drain_patched", False):
        return

    def _patched(self, tick_clock, wait_clock):
        nc = self.nc
        probe = nc.sync.nop()
        wait_clock.add_sem_waits(
            probe.ins, ScopedClock({None: tick_clock.global_clock})
        )
        si = probe.ins.sync_info
        waits = list(si.on_wait) if si and si.on_wait else []
        if len(waits) > 1:
            si.on_wait.clear()
            si.on_wait.append(waits[0])
            for w in waits[1:]:
                extra = nc.sync.nop()
                esi = extra.ins.sync_info
                if esi is None:
                    extra.ins.sync_info = mybir.SyncInfo(
                        on_wait=[w], on_update=[]
                    )
                else:
                    esi.on_wait.append(w)
        nc.sync.drain()
        nc.all_engine_barrier()
        assert self.sems is not None
        popped = nc._tile_sem_poison_stack.pop()
        assert popped is self._sem_poison
        nc.clear_and_free_semaphores(list(self.sems.allocated().values()))
        nc.all_engine_barrier()

    tile.TileContext._drain_and_barrier = _patched
    tile.TileContext._ant_drain_patched = True


def _split_excess_waits(nc, limit=1):
    """This container's walrus build rejects instructions carrying more
    than a couple of semaphore waits ("Too many sync wait commands";
    the limit varies per opcode — Matmult fails at 2). Move excess
    waits onto same-engine NoOps inserted immediately before the
    instruction; per-engine program order keeps semantics identical."""
    uid = 0
    for f in nc.m.functions:
        for blk in f.blocks:
            insts = blk.instructions
            out = []
            for inst in insts:
                si = inst.sync_info
                if si is not None and si.on_wait and len(si.on_wait) > limit:
                    waits = list(si.on_wait)
                    excess, keep = waits[:-limit], waits[-limit:]
                    for i in range(0, len(excess), limit):
                        nop = mybir.InstNoOp(
                            name=f"WSPLIT-{uid}", ins=[], outs=[]
                        )
                        uid += 1
                        nop.engine = inst.engine
                        nop.sync_info = mybir.SyncInfo(
                            on_wait=excess[i : i + limit], on_update=[]
                        )
                        out.append(nop)
                    si.on_wait.clear()
                    si.on_wait.extend(keep)
                out.append(inst)
            insts[:] = out


def build_nc(split_waits=True):
    _patch_tile_tail_drain()
    nc = bass.Bass()
    xT = nc.declare_dram_parameter("xT", [K, B], DT, isOutput=False)
    wT = nc.declare_dram_parameter("wT", [K, D_SH], DT, isOutput=False)
    bin_ = nc.declare_dram_parameter("bin", [1, D_SH], DT, isOutput=False)
    wout = nc.declare_dram_parameter("wout", [1, D_SH], DT, isOutput=False)
    bout = nc.declare_dram_parameter("bout", [1, U_SH], DT, isOutput=False)
    out = nc.declare_dram_parameter("out", [B, U_SH], DT, isOutput=True)

    with tile.TileContext(nc) as tc, ExitStack() as ctx:
        const = ctx.enter_context(tc.tile_pool(name="const", bufs=1))
        wpool = ctx.enter_context(tc.tile_pool(name="wpool", bufs=2))
        hpool = ctx.enter_context(tc.tile_pool(name="hpool", bufs=4))
        epool = ctx.enter_context(tc.tile_pool(name="epool", bufs=4))
        opool = ctx.enter_context(tc.tile_pool(name="opool", bufs=2))
        pspool = ctx.enter_context(
            tc.tile_pool(name="pspool", bufs=4, space="PSUM")
        )

        # ---- constants ----
        xt_sb = const.tile([128, KT, B], DT)
        nc.sync.dma_start(xt_sb[:], xT.rearrange("(t p) b -> p t b", p=128))

        bin_bc = const.tile([128, D_SH], DT)
        nc.sync.dma_start(bin_bc[:], bin_[0:1, :].broadcast_to([128, D_SH]))

        wout_bc = const.tile([128, D_SH], DT)
        nc.sync.dma_start(wout_bc[:], wout[0:1, :].broadcast_to([128, D_SH]))

        bout_bc = const.tile([128, U_SH], DT)
        nc.sync.dma_start(bout_bc[:], bout[0:1, :].broadcast_to([128, U_SH]))

        m_t = [const.tile([128, U_SH], DT, name=f"m{b}") for b in range(NB)]
        s_t = [const.tile([128, U_SH], DT, name=f"s{b}") for b in range(NB)]

        # ---- main stream: matmul + chunked epilogue ----
        for dc in range(DC):
            dsl = slice(dc * DC_W, (dc + 1) * DC_W)
            usl = slice(dc * UC, (dc + 1) * UC)
            w_sb = wpool.tile([128, KT, DC_W], DT, name="w_sb")
            nc.sync.dma_start(
                w_sb[:], wT[:, dsl].rearrange("(t p) d -> p t d", p=128)
            )
            for b in range(NB):
                bsl = slice(b * 128, (b + 1) * 128)
                ps = pspool.tile([128, DC_W], DT, name="ps")
                for k in range(KT):
                    nc.tensor.matmul(
                        ps[:],
                        xt_sb[:, k, bsl],
                        w_sb[:, k, :],
                        start=(k == 0),
                        stop=(k == KT - 1),
                    )
                hc = hpool.tile([128, DC_W], DT, name="hc")
                nc.vector.tensor_add(hc[:], ps[:], bin_bc[:, dsl])

                hc3 = hc.rearrange("p (u e) -> p u e", e=DPN)
                nc.vector.reduce_max(m_t[b][:, usl], hc3, axis=AX)
                mb3 = (
                    m_t[b][:, usl]
                    .unsqueeze(2)
                    .broadcast_to([128, UC, DPN])
                )
                eqc = epool.tile([128, DC_W], DT, name="eqc")
                nc.vector.tensor_tensor(
                    eqc.rearrange("p (u e) -> p u e", e=DPN),
                    hc3,
                    mb3,
                    op=mybir.AluOpType.is_equal,
                )
                tcw = epool.tile([128, DC_W], DT, name="tcw")
                nc.vector.tensor_mul(tcw[:], eqc[:], wout_bc[:, dsl])
                nc.vector.reduce_sum(
                    s_t[b][:, usl],
                    tcw.rearrange("p (u e) -> p u e", e=DPN),
                    axis=AX,
                )

        # ---- finale ----
        for b in range(NB):
            o1 = opool.tile([128, U_SH], DT, name="o1")
            nc.vector.tensor_mul(o1[:], m_t[b][:], s_t[b][:])
            o2 = opool.tile([128, U_SH], DT, name="o2")
            nc.vector.tensor_add(o2[:], o1[:], bout_bc[:])
            nc.sync.dma_start(out[b * 128 : (b + 1) * 128, :], o2[:])

    if split_waits:
        _split_excess_waits(nc)
    return nc


def make_in_maps(x, w_in, b_in, w_out, b_out):
    xT = np.ascontiguousarray(x.T.astype(np.float32, copy=False))
    w_inT = np.ascontiguousarray(w_in.T.astype(np.float32, copy=False))
    in_maps = []
    for c in range(N_CORES):
        dsl = slice(c * D_SH, (c + 1) * D_SH)
        usl = slice(c * U_SH, (c + 1) * U_SH)
        in_maps.append(
            {
                "xT": xT,
                "wT": np.ascontiguousarray(w_inT[:, dsl]),
                "bin": np.ascontiguousarray(
                    b_in[dsl].reshape(1, D_SH).astype(np.float32, copy=False)
                ),
                "wout": np.ascontiguousarray(
                    w_out[usl].reshape(1, D_SH).astype(np.float32, copy=False)
                ),
                "bout": np.ascontiguousarray(
                    b_out[usl].reshape(1, U_SH).astype(np.float32, copy=False)
                ),
            }
        )
    return in_maps


def run(in_maps, trace=False, **kw):
    nc = build_nc()
    return run_bass_kernel_spmd(
        nc, in_maps, list(range(N_CORES)), trace=trace, **kw
    )


def kernel(x, w_in, b_in, w_out, b_out):
    in_maps = make_in_maps(x, w_in, b_in, w_out, b_out)
    res = run(in_maps, trace=False)
    return np.concatenate(
        [res.results[c]["out"] for c in range(N_CORES)], axis=1
    )


# revision 7
# speedup vs baseline: 1.0901x; 1.0901x over previous
"""Trainium2 Bass kernel for nn_DendriteLayer (topk_masking).

Computation (see reference):
    h  = x @ w_in.T + b_in                    # [B, N_DEND]
    h3 = h.reshape(B, OUT_DIM, DPN)
    out[b,u] = h3[b,u,argmax_d h3[b,u,:]] * w_out[u, argmax_d] + b_out[u]

Sharding: OUT_DIM (and its DPN dendrite groups) split across 8 cores;
x replicated; no cross-core communication. Each core computes a
[B, OUT_DIM/8] slice of the output.

Device layout: batch on partitions, dendrites on the free dim, so the
per-unit max over DPN=16 consecutive dendrites is a free-dim segmented
reduce on the vector engine. w_in is pre-transposed on host to
[IN_DIM, N_DEND] so the contraction dim lands on partitions with
contiguous DMA rows.
"""

import numpy as np

import concourse.bass as bass
import concourse.mybir as mybir
from concourse import tile
from concourse.bass_utils import run_bass_kernel_spmd
from concourse.vector_clock import ScopedClock
from contextlib import ExitStack

# Problem shapes (hardcoded per contract).
B = 256          # batch
K = 1024         # in_dim
OUT_DIM = 2048
DPN = 16
N_CORES = 8
D_SH = (OUT_DIM // N_CORES) * DPN   # 4096 dendrites per core
U_SH = OUT_DIM // N_CORES           # 256 units per core
KT = K // 128                       # 8 k-tiles
DC_W = 512                          # dendrite chunk width (PSUM bank)
DC = D_SH // DC_W                   # 8 chunks
UC = DC_W // DPN                    # 32 units per chunk
NB = B // 128                       # 2 batch tiles
DT = mybir.dt.float32
AX = mybir.AxisListType.X


def _patch_tile_tail_drain():
    """Workaround: this container's walrus build rejects >2 semaphore
    waits on one InstDrain ("Too many sync wait commands"). Move the
    TileContext tail-drain waits onto individual SP NOPs (one wait
    each); SP program order keeps the drain equivalent."""
    if getattr(tile.TileContext, "_ant_drain_patched", False):
        return

    def _patched(self, tick_clock, wait_clock):
        nc = self.nc
        probe = nc.sync.nop()
        wait_clock.add_sem_waits(
            probe.ins, ScopedClock({None: tick_clock.global_clock})
        )
        si = probe.ins.sync_info
        waits = list(si.on_wait) if si and si.on_wait else []
        if len(waits) > 1:
            si.on_wait.clear()
            si.on_wait.append(waits[0])
            for w in waits[1:]:
                extra = nc.sync.nop()
                esi = extra.ins.sync_info
                if esi is None:
                    extra.ins.sync_info = mybir.SyncInfo(
                        on_wait=[w], on_update=[]
                    )
                else:
                    esi.on_wait.append(w)
        nc.sync.drain()
        nc.all_engine_barrier()
        assert self.sems is not None
        popped = nc._tile_sem_poison_stack.pop()
        assert popped is self._sem_poison
        nc.clear_and_free_semaphores(list(self.sems.allocated().values()))
        nc.all_engine_barrier()

    tile.TileContext._drain_and_barrier = _patched
    tile.TileContext._ant_drain_patched = True


def _split_excess_waits(nc, limit=1):
    """This container's walrus build rejects instructions carrying more
    than a couple of semaphore waits ("Too many sync wait commands";
    the limit varies per opcode — Matmult fails at 2). Move excess
    waits onto same-engine NoOps inserted immediately before the
    instruction; per-engine program order keeps semantics identical."""
    uid = 0
    for f in nc.m.functions:
        for blk in f.blocks:
            insts = blk.instructions
            out = []
            for inst in insts:
                si = inst.sync_info
                if si is not None and si.on_wait and len(si.on_wait) > limit:
                    waits = list(si.on_wait)
                    excess, keep = waits[:-limit], waits[-limit:]
                    for i in range(0, len(excess), limit):
                        nop = mybir.InstNoOp(
                            name=f"WSPLIT-{uid}", ins=[], outs=[]
                        )
                        uid += 1
                        nop.engine = inst.engine
                        nop.sync_info = mybir.SyncInfo(
                            on_wait=excess[i : i + limit], on_update=[]
                        )
                        out.append(nop)
                    si.on_wait.clear()
                    si.on_wait.extend(keep)
                out.append(inst)
            insts[:] = out


def build_nc(split_waits=True):
    _patch_tile_tail_drain()
    nc = bass.Bass()
    xT = nc.declare_dram_parameter("xT", [K, B], DT, isOutput=False)
    wT = nc.declare_dram_parameter("wT", [K, D_SH], DT, isOutput=False)
    bin_ = nc.declare_dram_parameter("bin", [1, D_SH], DT, isOutput=False)
    wout = nc.declare_dram_parameter("wout", [1, D_SH], DT, isOutput=False)
    bout = nc.declare_dram_parameter("bout", [1, U_SH], DT, isOutput=False)
    out = nc.declare_dram_parameter("out", [B, U_SH], DT, isOutput=True)

    with tile.TileContext(nc) as tc, ExitStack() as ctx:
        const = ctx.enter_context(tc.tile_pool(name="const", bufs=1))
        wpool = ctx.enter_context(tc.tile_pool(name="wpool", bufs=3))
        hpool = ctx.enter_context(tc.tile_pool(name="hpool", bufs=6))
        epool = ctx.enter_context(tc.tile_pool(name="epool", bufs=6))
        opool = ctx.enter_context(tc.tile_pool(name="opool", bufs=2))
        pspool = ctx.enter_context(
            tc.tile_pool(name="pspool", bufs=8, space="PSUM")
        )

        # ---- x (stationary) first so the PE can start ASAP ----
        xt_sb = const.tile([128, KT, B], DT)
        nc.sync.dma_start(xt_sb[:], xT.rearrange("(t p) b -> p t b", p=128))

        # ---- w prefetch: 2 chunks ahead of the broadcast constants so
        # the matmul stream is never starved at the head ----
        w_tiles = {}

        def load_w(dc):
            dsl = slice(dc * DC_W, (dc + 1) * DC_W)
            w_sb = wpool.tile([128, KT, DC_W], DT, name="w_sb")
            nc.sync.dma_start(
                w_sb[:], wT[:, dsl].rearrange("(t p) d -> p t d", p=128)
            )
            w_tiles[dc] = w_sb

        load_w(0)
        load_w(1)

        # ---- broadcast constants (after the first two w chunks) ----
        bin_bc = const.tile([128, D_SH], DT)
        nc.sync.dma_start(bin_bc[:], bin_[0:1, :].broadcast_to([128, D_SH]))

        wout_bc = const.tile([128, D_SH], DT)
        nc.sync.dma_start(wout_bc[:], wout[0:1, :].broadcast_to([128, D_SH]))

        bout_bc = const.tile([128, U_SH], DT)
        nc.sync.dma_start(bout_bc[:], bout[0:1, :].broadcast_to([128, U_SH]))

        m_t = [const.tile([128, U_SH], DT, name=f"m{b}") for b in range(NB)]
        s_t = [const.tile([128, U_SH], DT, name=f"s{b}") for b in range(NB)]

        # ---- main stream: matmul + chunked epilogue ----
        for dc in range(DC):
            if dc + 2 < DC:
                load_w(dc + 2)
            dsl = slice(dc * DC_W, (dc + 1) * DC_W)
            usl = slice(dc * UC, (dc + 1) * UC)
            w_sb = w_tiles.pop(dc)
            for b in range(NB):
                bsl = slice(b * 128, (b + 1) * 128)
                ps = pspool.tile([128, DC_W], DT, name="ps")
                for k in range(KT):
                    nc.tensor.matmul(
                        ps[:],
                        xt_sb[:, k, bsl],
                        w_sb[:, k, :],
                        start=(k == 0),
                        stop=(k == KT - 1),
                    )
                hc = hpool.tile([128, DC_W], DT, name="hc")
                nc.vector.tensor_add(hc[:], ps[:], bin_bc[:, dsl])

                hc3 = hc.rearrange("p (u e) -> p u e", e=DPN)
                nc.vector.reduce_max(m_t[b][:, usl], hc3, axis=AX)
                mb3 = (
                    m_t[b][:, usl]
                    .unsqueeze(2)
                    .broadcast_to([128, UC, DPN])
                )
                eqc = epool.tile([128, DC_W], DT, name="eqc")
                nc.vector.tensor_tensor(
                    eqc.rearrange("p (u e) -> p u e", e=DPN),
                    hc3,
                    mb3,
                    op=mybir.AluOpType.is_equal,
                )
                tcw = epool.tile([128, DC_W], DT, name="tcw")
                nc.vector.tensor_mul(tcw[:], eqc[:], wout_bc[:, dsl])
                nc.vector.reduce_sum(
                    s_t[b][:, usl],
                    tcw.rearrange("p (u e) -> p u e", e=DPN),
                    axis=AX,
                )

        # ---- finale ----
        for b in range(NB):
            o1 = opool.tile([128, U_SH], DT, name="o1")
            nc.vector.tensor_mul(o1[:], m_t[b][:], s_t[b][:])
            o2 = opool.tile([128, U_SH], DT, name="o2")
            nc.vector.tensor_add(o2[:], o1[:], bout_bc[:])
            nc.sync.dma_start(out[b * 128 : (b + 1) * 128, :], o2[:])

    if split_waits:
        _split_excess_waits(nc)
    return nc


def make_in_maps(x, w_in, b_in, w_out, b_out):
    xT = np.ascontiguousarray(x.T.astype(np.float32, copy=False))
    w_inT = np.ascontiguousarray(w_in.T.astype(np.float32, copy=False))
    in_maps = []
    for c in range(N_CORES):
        dsl = slice(c * D_SH, (c + 1) * D_SH)
        usl = slice(c * U_SH, (c + 1) * U_SH)
        in_maps.append(
            {
                "xT": xT,
                "wT": np.ascontiguousarray(w_inT[:, dsl]),
                "bin": np.ascontiguousarray(
                    b_in[dsl].reshape(1, D_SH).astype(np.float32, copy=False)
                ),
                "wout": np.ascontiguousarray(
                    w_out[usl].reshape(1, D_SH).astype(np.float32, copy=False)
                ),
                "bout": np.ascontiguousarray(
                    b_out[usl].reshape(1, U_SH).astype(np.float32, copy=False)
                ),
            }
        )
    return in_maps


def run(in_maps, trace=False, **kw):
    nc = build_nc()
    return run_bass_kernel_spmd(
        nc, in_maps, list(range(N_CORES)), trace=trace, **kw
    )


def kernel(x, w_in, b_in, w_out, b_out):
    in_maps = make_in_maps(x, w_in, b_in, w_out, b_out)
    res = run(in_maps, trace=False)
    return np.concatenate(
        [res.results[c]["out"] for c in range(N_CORES)], axis=1
    )


# revision 11
# speedup vs baseline: 1.1081x; 1.0165x over previous
"""Trainium2 Bass kernel for nn_DendriteLayer (topk_masking).

Computation (see reference):
    h  = x @ w_in.T + b_in                    # [B, N_DEND]
    h3 = h.reshape(B, OUT_DIM, DPN)
    out[b,u] = h3[b,u,argmax_d h3[b,u,:]] * w_out[u, argmax_d] + b_out[u]

Sharding: OUT_DIM (and its DPN dendrite groups) split across 8 cores;
x replicated; no cross-core communication. Each core computes a
[B, OUT_DIM/8] slice of the output.

Device layout: batch on partitions, dendrites on the free dim, so the
per-unit max over DPN=16 consecutive dendrites is a free-dim segmented
reduce on the vector engine. w_in is pre-transposed on host to
[IN_DIM, N_DEND] so the contraction dim lands on partitions with
contiguous DMA rows.
"""

import numpy as np

import concourse.bass as bass
import concourse.mybir as mybir
from concourse import tile
from concourse.bass_utils import run_bass_kernel_spmd
from concourse.vector_clock import ScopedClock
from contextlib import ExitStack

# Problem shapes (hardcoded per contract).
B = 256          # batch
K = 1024         # in_dim
OUT_DIM = 2048
DPN = 16
N_CORES = 8
D_SH = (OUT_DIM // N_CORES) * DPN   # 4096 dendrites per core
U_SH = OUT_DIM // N_CORES           # 256 units per core
KT = K // 128                       # 8 k-tiles
DC_W = 512                          # dendrite chunk width (PSUM bank)
DC = D_SH // DC_W                   # 8 chunks
UC = DC_W // DPN                    # 32 units per chunk
NB = B // 128                       # 2 batch tiles
DT = mybir.dt.float32
AX = mybir.AxisListType.X


def _patch_tile_tail_drain():
    """Workaround: this container's walrus build rejects >2 semaphore
    waits on one InstDrain ("Too many sync wait commands"). Move the
    TileContext tail-drain waits onto individual SP NOPs (one wait
    each); SP program order keeps the drain equivalent."""
    if getattr(tile.TileContext, "_ant_drain_patched", False):
        return

    def _patched(self, tick_clock, wait_clock):
        nc = self.nc
        probe = nc.sync.nop()
        wait_clock.add_sem_waits(
            probe.ins, ScopedClock({None: tick_clock.global_clock})
        )
        si = probe.ins.sync_info
        waits = list(si.on_wait) if si and si.on_wait else []
        if len(waits) > 1:
            si.on_wait.clear()
            si.on_wait.append(waits[0])
            for w in waits[1:]:
                extra = nc.sync.nop()
                esi = extra.ins.sync_info
                if esi is None:
                    extra.ins.sync_info = mybir.SyncInfo(
                        on_wait=[w], on_update=[]
                    )
                else:
                    esi.on_wait.append(w)
        nc.sync.drain()
        nc.all_engine_barrier()
        assert self.sems is not None
        popped = nc._tile_sem_poison_stack.pop()
        assert popped is self._sem_poison
        nc.clear_and_free_semaphores(list(self.sems.allocated().values()))
        nc.all_engine_barrier()

    tile.TileContext._drain_and_barrier = _patched
    tile.TileContext._ant_drain_patched = True


def _split_excess_waits(nc, limit=1):
    """This container's walrus build rejects instructions carrying more
    than a couple of semaphore waits ("Too many sync wait commands";
    the limit varies per opcode — Matmult fails at 2). Move excess
    waits onto same-engine NoOps inserted immediately before the
    instruction; per-engine program order keeps semantics identical."""
    uid = 0
    for f in nc.m.functions:
        for blk in f.blocks:
            insts = blk.instructions
            out = []
            for inst in insts:
                si = inst.sync_info
                if si is not None and si.on_wait and len(si.on_wait) > limit:
                    waits = list(si.on_wait)
                    excess, keep = waits[:-limit], waits[-limit:]
                    for i in range(0, len(excess), limit):
                        nop = mybir.InstNoOp(
                            name=f"WSPLIT-{uid}", ins=[], outs=[]
                        )
                        uid += 1
                        nop.engine = inst.engine
                        nop.sync_info = mybir.SyncInfo(
                            on_wait=excess[i : i + limit], on_update=[]
                        )
                        out.append(nop)
                    si.on_wait.clear()
                    si.on_wait.extend(keep)
                out.append(inst)
            insts[:] = out


def build_nc(split_waits=True):
    _patch_tile_tail_drain()
    nc = bass.Bass()
    xT = nc.declare_dram_parameter("xT", [K, B], DT, isOutput=False)
    wT = nc.declare_dram_parameter("wT", [K, D_SH], DT, isOutput=False)
    bin_ = nc.declare_dram_parameter("bin", [1, D_SH], DT, isOutput=False)
    wout = nc.declare_dram_parameter("wout", [1, D_SH], DT, isOutput=False)
    bout = nc.declare_dram_parameter("bout", [1, U_SH], DT, isOutput=False)
    out = nc.declare_dram_parameter("out", [B, U_SH], DT, isOutput=True)

    with tile.TileContext(nc) as tc, ExitStack() as ctx:
        const = ctx.enter_context(tc.tile_pool(name="const", bufs=1))
        wpool = ctx.enter_context(tc.tile_pool(name="wpool", bufs=3))
        hpool = ctx.enter_context(tc.tile_pool(name="hpool", bufs=6))
        epool = ctx.enter_context(tc.tile_pool(name="epool", bufs=6))
        opool = ctx.enter_context(tc.tile_pool(name="opool", bufs=4))
        pspool = ctx.enter_context(
            tc.tile_pool(name="pspool", bufs=8, space="PSUM")
        )

        # ---- x (stationary) on the scalar HWDGE ring, split in halves,
        # so it loads in parallel with the w stream on the sync ring ----
        xt_view = xT.rearrange("(t p) b -> p t b", p=128)
        xt_sb = const.tile([128, KT, B], DT)
        nc.scalar.dma_start(xt_sb[:, : KT // 2, :], xt_view[:, : KT // 2, :])
        nc.scalar.dma_start(xt_sb[:, KT // 2 :, :], xt_view[:, KT // 2 :, :])

        # ---- w stream (sync ring only); first chunk split in k-halves
        # so the PE can start on k0-3 at half-load ----
        w_tiles = {}

        def load_w(dc, split=False):
            dsl = slice(dc * DC_W, (dc + 1) * DC_W)
            wv = wT[:, dsl].rearrange("(t p) d -> p t d", p=128)
            w_sb = wpool.tile([128, KT, DC_W], DT, name="w_sb")
            if split:
                nc.sync.dma_start(
                    w_sb[:, : KT // 2, :], wv[:, : KT // 2, :]
                )
                nc.sync.dma_start(
                    w_sb[:, KT // 2 :, :], wv[:, KT // 2 :, :]
                )
            else:
                nc.sync.dma_start(w_sb[:], wv)
            w_tiles[dc] = w_sb

        load_w(0, split=True)
        load_w(1)

        # ---- broadcast constants (scalar ring, behind x) ----
        bin_bc = const.tile([128, D_SH], DT)
        nc.scalar.dma_start(bin_bc[:], bin_[0:1, :].broadcast_to([128, D_SH]))

        wout_bc = const.tile([128, D_SH], DT)
        nc.scalar.dma_start(
            wout_bc[:], wout[0:1, :].broadcast_to([128, D_SH])
        )

        bout_bc = const.tile([128, U_SH], DT)
        nc.scalar.dma_start(
            bout_bc[:], bout[0:1, :].broadcast_to([128, U_SH])
        )

        m_t = [const.tile([128, U_SH], DT, name=f"m{b}") for b in range(NB)]

        # ---- main stream: matmul + chunked epilogue ----
        for dc in range(DC):
            if dc + 2 < DC:
                load_w(dc + 2)
            dsl = slice(dc * DC_W, (dc + 1) * DC_W)
            usl = slice(dc * UC, (dc + 1) * UC)
            w_sb = w_tiles.pop(dc)
            for b in range(NB):
                bsl = slice(b * 128, (b + 1) * 128)
                ps = pspool.tile([128, DC_W], DT, name="ps")
                for k in range(KT):
                    nc.tensor.matmul(
                        ps[:],
                        xt_sb[:, k, bsl],
                        w_sb[:, k, :],
                        start=(k == 0),
                        stop=(k == KT - 1),
                    )
                hc = hpool.tile([128, DC_W], DT, name="hc")
                nc.vector.tensor_add(hc[:], ps[:], bin_bc[:, dsl])

                hc3 = hc.rearrange("p (u e) -> p u e", e=DPN)
                nc.vector.reduce_max(m_t[b][:, usl], hc3, axis=AX)
                mb3 = (
                    m_t[b][:, usl]
                    .unsqueeze(2)
                    .broadcast_to([128, UC, DPN])
                )
                eqc = epool.tile([128, DC_W], DT, name="eqc")
                nc.vector.tensor_tensor(
                    eqc.rearrange("p (u e) -> p u e", e=DPN),
                    hc3,
                    mb3,
                    op=mybir.AluOpType.is_equal,
                )
                tcw = epool.tile([128, DC_W], DT, name="tcw")
                nc.vector.tensor_mul(tcw[:], eqc[:], wout_bc[:, dsl])
                sc = epool.tile([128, UC], DT, name="sc")
                nc.vector.reduce_sum(
                    sc[:],
                    tcw.rearrange("p (u e) -> p u e", e=DPN),
                    axis=AX,
                )
                # finale per chunk: out slice = m*s + bout, streamed out
                o1 = opool.tile([128, UC], DT, name="o1")
                nc.vector.tensor_mul(o1[:], m_t[b][:, usl], sc[:])
                o2 = opool.tile([128, UC], DT, name="o2")
                nc.vector.tensor_add(o2[:], o1[:], bout_bc[:, usl])
                nc.scalar.dma_start(
                    out[b * 128 : (b + 1) * 128, usl], o2[:]
                )

    if split_waits:
        _split_excess_waits(nc)
    return nc


def make_in_maps(x, w_in, b_in, w_out, b_out):
    xT = np.ascontiguousarray(x.T.astype(np.float32, copy=False))
    w_inT = np.ascontiguousarray(w_in.T.astype(np.float32, copy=False))
    in_maps = []
    for c in range(N_CORES):
        dsl = slice(c * D_SH, (c + 1) * D_SH)
        usl = slice(c * U_SH, (c + 1) * U_SH)
        in_maps.append(
            {
                "xT": xT,
                "wT": np.ascontiguousarray(w_inT[:, dsl]),
                "bin": np.ascontiguousarray(
                    b_in[dsl].reshape(1, D_SH).astype(np.float32, copy=False)
                ),
                "wout": np.ascontiguousarray(
                    w_out[usl].reshape(1, D_SH).astype(np.float32, copy=False)
                ),
                "bout": np.ascontiguousarray(
                    b_out[usl].reshape(1, U_SH).astype(np.float32, copy=False)
                ),
            }
        )
    return in_maps


def run(in_maps, trace=False, **kw):
    nc = build_nc()
    return run_bass_kernel_spmd(
        nc, in_maps, list(range(N_CORES)), trace=trace, **kw
    )


def kernel(x, w_in, b_in, w_out, b_out):
    in_maps = make_in_maps(x, w_in, b_in, w_out, b_out)
    res = run(in_maps, trace=False)
    return np.concatenate(
        [res.results[c]["out"] for c in range(N_CORES)], axis=1
    )
